# revision 1
# baseline (speedup 1.0000x reference)
"""Tensor-parallel transformer block (post-LN, BERT-style) on 8 TRN2 NeuronCores.

Sharding: 8 cores = 4 batches x 2 head-groups. Core c=(b,g) computes, for batch b:
  - Q/K/V projections + attention for its 8 heads (Megatron column-parallel),
  - its partial of the attention output projection (row-parallel),
  - pairwise ReduceScatter over the 2 cores sharing batch b -> core owns 1024
    tokens,
  - LN1, full FFN (token-parallel, whole W1/W2), LN2 for its 1024 tokens.
Host concatenates the 8 [1024, 1024] output slices.

Layouts keep activations transposed ([feature, token]) so every matmul uses
weights in natural layout; softmax row-sums come from a ones-column appended to
V; mask and 1/sqrt(hd) fold into the exp/projection epilogues.
"""

import os
import numpy as np

import concourse.bacc as bacc
import concourse.bass as bass
import concourse.tile as tile
import concourse.mybir as mybir
from concourse.bass_utils import run_bass_kernel_spmd

P = 128
F32 = mybir.dt.float32
BF16 = mybir.dt.bfloat16
AF = mybir.ActivationFunctionType
ALU = mybir.AluOpType

REPLICA_GROUPS = [[0, 1], [2, 3], [4, 5], [6, 7]]


def build_block(nc, *, S, H, NH_core, FF, eps=1e-12, flags=None, dbg=False, prefix="", stage=4, shared=None):
    """Emit the SPMD program for one core. flags: set of optional-input names
    among {mask, bq, bk, bv, bo, b1, b2, ln1_g, ln1_b, ln2_g, ln2_b} that are
    actually present (nonzero / non-one)."""
    flags = flags or set()
    HD = 64
    D3 = NH_core * HD          # per-core qkv width
    HT = H // P                # x-feature subtiles
    ST = S // P                # token tiles (full batch)
    S2 = S // 2                # tokens owned after reduce-scatter
    ST2 = ST // 2
    DT3 = max(1, D3 // P)      # qkv-feature subtiles
    QC = min(512, S)           # q chunk
    NQC = S // QC
    TC = min(512, S)           # token chunk in projections
    NTC = S // TC
    KT = ST                    # k tiles
    KG = min(4, KT)            # k-tiles per exp batch
    NKG = KT // KG
    HOC = min(512, H)
    NHOC = H // HOC
    NFQ = 4 if FF % (4 * P) == 0 else 2   # stream FFN weights in quarters
    FQ = FF // NFQ
    FTQ = FQ // P
    TC2 = min(512, S2)
    NTC2 = S2 // TC2
    assert D3 % P == 0 or NH_core == 2

    def pn(n):
        return f"{prefix}{n}"

    def param(name, shape, dt=F32):
        if shared is not None and name in shared:
            return shared[name]
        h = nc.declare_dram_parameter(name if shared is not None else pn(name),
                                      list(shape), dt, isOutput=False)
        if shared is not None:
            shared[name] = h
        return h

    x = param("x", [S, H], BF16)
    xh = param("xh", [S2, H])
    wq = param("wq", [H, D3], BF16)
    wk = param("wk", [H, D3], BF16)
    wv = param("wv", [H, D3], BF16)
    wo = param("wo", [D3, H], BF16)
    w1 = param("w1", [H, FF], BF16)
    w2 = param("w2", [FF, H], BF16)
    opt = {}
    for name, shape in [("mask", [S]), ("bq", [D3]), ("bk", [D3]), ("bv", [D3]),
                        ("bo", [H]), ("b1", [FF]), ("b2", [H]),
                        ("ln1_g", [H]), ("ln1_b", [H]),
                        ("ln2_g", [H]), ("ln2_b", [H])]:
        if name in flags:
            opt[name] = param(name, shape)
    out_ext = nc.declare_dram_parameter(pn("out"), [S2, H], F32, isOutput=True)
    dbg_ext = {}
    if dbg:
        for nm, shape in [("d_xT", [P, HT * S]), ("d_qT", [P, DT3 * S]),
                          ("d_kT", [P, DT3 * S]),
                          ("d_v", [P, KT * NH_core * (HD + 1)]),
                          ("d_probs", [P, KT * QC]),
                          ("d_rowsum", [1, NH_core * S]),
                          ("d_ctxT", [P, DT3 * S]),
                          ("d_ctxTu", [P, DT3 * S]),
                          ("d_rinvrep", [P, NH_core * S]),
                          ("d_rinv", [1, NH_core * S]),
                          ("d_attn", [P, ST * H]),
                          ("d_rsout", [S2, H]),
                          ("d_x1", [P, ST2 * H]),
                          ("d_x1T", [P, HT * S2]),
                          ("d_out2", [P, ST2 * H])]:
            dbg_ext[nm] = nc.declare_dram_parameter(pn(nm), shape, F32, isOutput=True)

    def dump(nm, ap):
        if dbg:
            n = 1
            for d in ap.shape[1:]:
                n *= d
            tmp = dram.tile(list(ap.shape), F32, tag=f"dump_{nm}")
            nc.gpsimd.dma_start(tmp, ap)
            nc.gpsimd.dma_start(
                dbg_ext[nm][:ap.shape[0], :n],
                tmp[:].rearrange(" ".join(f"a{i}" for i in range(len(ap.shape)))
                                 + " -> a0 (" +
                                 " ".join(f"a{i}" for i in range(1, len(ap.shape)))
                                 + ")"))

    with (
        tile.TileContext(nc) as tc,
        tc.tile_pool(name=pn("singles"), bufs=1) as singles,
        tc.tile_pool(name=pn("dram"), bufs=1, space="DRAM") as dram,
    ):
        eps_sb = singles.tile([P, 1], F32)
        nc.vector.memset(eps_sb, eps)
        mask_sb = None
        if "mask" in flags:
            mask_sb = singles.tile([P, KT], F32)
            nc.gpsimd.dma_start(mask_sb, opt["mask"].rearrange("(a p) -> p a", p=P))
        # per-partition bias strips
        def col_strip(name, n):
            if name not in flags:
                return None
            t = singles.tile([P, n // P], F32, tag=f"strip_{name}")
            nc.gpsimd.dma_start(t, opt[name].rearrange("(a p) -> p a", p=P))
            return t
        bq_sb = col_strip("bq", D3)
        bk_sb = col_strip("bk", D3)
        b1_sb = col_strip("b1", FF)
        # partition-replicated rows (for free-dim adds)
        def rep_row(name, n):
            if name not in flags:
                return None
            t = singles.tile([P, n], F32, tag=f"rep_{name}")
            src = opt[name][:]
            bcast = bass.AP(tensor=src.tensor, offset=src.offset,
                            ap=[[0, P]] + list(src.ap))
            nc.gpsimd.dma_start(t, bcast)
            return t
        bv_sb = rep_row("bv", D3)
        bo_sb = rep_row("bo", H)
        b2_sb = rep_row("b2", H)
        ln1g_sb = rep_row("ln1_g", H)
        ln1b_sb = rep_row("ln1_b", H)
        ln2g_sb = rep_row("ln2_g", H)
        ln2b_sb = rep_row("ln2_b", H)

        rs_in = dram.tile([S, H], BF16)
        rs_out = dram.tile([S2, H], BF16)

        # ---------------- phase 1: x -> xT (bf16), QKV projections ----------
        with tc.tile_pool(name=pn("attn_keep"), bufs=1) as keep:
            qT = keep.tile([P, DT3, S], BF16)
            kTt = keep.tile([P, DT3, S], BF16)
            v_sb = keep.tile([P, KT, NH_core, HD + 1], BF16)
            ctxT = keep.tile([P, DT3, S], BF16)
            rowsum_dram = dram.tile([NH_core, S], BF16)
            nc.vector.memset(v_sb[:, :, :, HD:HD + 1], 1.0)

            with (
                tc.tile_pool(name="xT", bufs=1) as xTp,
                tc.tile_pool(name=pn("wqkv"), bufs=1) as wqkvp,
                tc.tile_pool(name=pn("qkv_ps"), bufs=4, space="PSUM") as qps,
            ):
                xT = xTp.tile([P, HT, S], BF16)

                def rsink(*aps):
                    sk = xTp.tile([P, 1], F32, tag="rsink")
                    for i, ap in enumerate(aps):
                        flat = ap.rearrange(
                            " ".join(f"a{j}" for j in range(len(ap.shape)))
                            + " -> a0 ("
                            + " ".join(f"a{j}" for j in range(1, len(ap.shape)))
                            + ")")
                        nc.vector.tensor_reduce(
                            sk, flat, mybir.AxisListType.X, ALU.add)
                        nc.sync.dma_start(out_ext[0:P, i:i + 1], sk)

                with tc.tile_pool(name=pn("xbf"), bufs=1) as xbfp:
                    x_bf = xbfp.tile([P, ST, H], BF16)
                    for tt in range(ST):
                        nc.sync.dma_start(x_bf[:, tt, :],
                                          x[tt * P:(tt + 1) * P, :])
                    if stage == 11:
                        rsink(x_bf[:])
                        return
                    for tt in range(ST):
                        nc.sync.dma_start_transpose(
                            xT[:, :, tt * P:(tt + 1) * P], x_bf[:, tt, :])
                if stage == 12:
                    rsink(xT[:])
                    return

                dump("d_xT", xT)
                wq_sb = wqkvp.tile([P, HT, D3], BF16)
                wk_sb = wqkvp.tile([P, HT, D3], BF16)
                wv_sb = wqkvp.tile([P, HT, D3], BF16)
                for w_ext, w_t in [(wq, wq_sb), (wk, wk_sb), (wv, wv_sb)]:
                    nc.sync.dma_start(
                        w_t, w_ext.rearrange("(a p) d -> p a d", p=P))
                if stage == 13:
                    rsink(wq_sb[:], wk_sb[:], wv_sb[:], xT[:])
                    return

                # qT / kT: [D3, S] feature-major
                for which, w_t, dest, b_t in [(0, wq_sb, qT, bq_sb),
                                              (1, wk_sb, kTt, bk_sb)]:
                    scale = 0.125 if which == 0 else 1.0
                    for dt in range(DT3):
                        for tci in range(NTC):
                            ps = qps.tile([P, TC], F32, tag="qk")
                            for ht in range(HT):
                                nc.tensor.matmul(
                                    ps,
                                    w_t[:, ht, dt * P:(dt + 1) * P],
                                    xT[:, ht, tci * TC:(tci + 1) * TC],
                                    start=(ht == 0), stop=(ht == HT - 1))
                            d_sl = dest[:, dt, tci * TC:(tci + 1) * TC]
                            if b_t is not None:
                                nc.vector.tensor_scalar(
                                    d_sl, ps, b_t[:, dt:dt + 1], scale,
                                    ALU.add, ALU.mult)
                            elif scale != 1.0:
                                nc.vector.tensor_scalar_mul(d_sl, ps, scale)
                            else:
                                nc.vector.tensor_copy(d_sl, ps)

                if stage == 14:
                    rsink(qT[:], kTt[:])
                    return
                # V: [S, D3] token-major (natural), +bias, ones col at HD
                for tt in range(ST):
                    ps = qps.tile([P, D3], F32, tag="v")
                    for ht in range(HT):
                        nc.tensor.matmul(
                            ps, xT[:, ht, tt * P:(tt + 1) * P],
                            wv_sb[:, ht, :],
                            start=(ht == 0), stop=(ht == HT - 1))
                    if bv_sb is not None:
                        nc.vector.tensor_tensor(ps, ps, bv_sb[:, :D3], ALU.add)
                    nc.vector.tensor_copy(
                        v_sb[:, tt, :, 0:HD],
                        ps.rearrange("p (nh hd) -> p nh hd", hd=HD))

            dump("d_qT", qT)
            dump("d_kT", kTt)
            dump("d_v", v_sb)
            # ---------------- phase 2: attention --------------------------
            with (
                tc.tile_pool(name=pn("probs"), bufs=2) as probsp,
                tc.tile_pool(name=pn("stage"), bufs=3) as stagep,
                tc.tile_pool(name=pn("rrep"), bufs=1) as rrepp,
                tc.tile_pool(name=pn("attn_sb"), bufs=1) as attnp,
                tc.tile_pool(name=pn("wo"), bufs=1) as wop,
                tc.tile_pool(name=pn("sc_ps"), bufs=1, space="PSUM") as scp,
                tc.tile_pool(name=pn("ctx_ps"), bufs=2, space="PSUM") as ctxp,
                tc.tile_pool(name=pn("wo_ps"), bufs=2, space="PSUM") as wops,
            ):
                for h in range(NH_core if stage >= 2 else 0):
                    hp, hs = divmod(h, 2)
                    hs *= HD
                    for qc in range(NQC):
                        q_sl = slice(qc * QC, (qc + 1) * QC)
                        probs = probsp.tile([P, KT, QC], BF16)
                        for kg in range(NKG):
                            ps_s = scp.tile([P, KG, QC], F32)
                            for j in range(KG):
                                kt = kg * KG + j
                                nc.tensor.matmul(
                                    ps_s[:, j, :],
                                    kTt[hs:hs + HD, hp, kt * P:(kt + 1) * P],
                                    qT[hs:hs + HD, hp, q_sl],
                                    start=True, stop=True)
                            if mask_sb is not None:
                                mvw = mask_sb[:, kg * KG:(kg + 1) * KG, None]
                                nc.vector.tensor_tensor(
                                    ps_s, ps_s,
                                    mvw.to_broadcast((P, KG, QC)), ALU.add)
                            nc.scalar.activation(
                                probs[:, kg * KG:(kg + 1) * KG, :], ps_s,
                                AF.Exp)
                        ps_c = ctxp.tile([P, QC], F32)
                        for kt in range(KT):
                            nc.tensor.matmul(
                                ps_c[0:HD + 1, :],
                                v_sb[:, kt, h, :],
                                probs[:, kt, :],
                                start=(kt == 0), stop=(kt == KT - 1))
                        if h == 1 and qc == 0:
                            dump("d_probs", probs)
                        cs = stagep.tile([P, QC], BF16)
                        nc.vector.tensor_copy(cs[0:HD + 1, :], ps_c[0:HD + 1, :])
                        nc.sync.dma_start(ctxT[hs:hs + HD, hp, q_sl],
                                          cs[0:HD, :])
                        nc.sync.dma_start(rowsum_dram[h:h + 1, q_sl],
                                          cs[HD:HD + 1, :])

                dump("d_ctxTu", ctxT)
                if stage < 3:
                    sink = stagep.tile([P, S], F32, tag="sink")
                    nc.vector.tensor_copy(sink[:, 0:S], qT[:, 0, :])
                    nc.vector.tensor_tensor(sink[:, 0:S], sink[:, 0:S],
                                            kTt[:, 0, :], ALU.add)
                    if stage >= 2:
                        nc.vector.tensor_tensor(sink[:, 0:S], sink[:, 0:S],
                                                ctxT[:, 0, :], ALU.add)
                    nc.vector.tensor_tensor(
                        sink[:, 0:S // 2], sink[:, 0:S // 2],
                        v_sb[:].rearrange("p a b c -> p (a b c)")[:, 0:S // 2],
                        ALU.add)
                    nc.sync.dma_start(out_ext[0:P, 0:H], sink[:, 0:H])
                    return
                # normalize ctx by broadcasted 1/rowsum (spread the row over
                # all partitions: 1-wide tiles waste a full partition-row)
                J = NH_core * S // P
                rs_sprd = rrepp.tile([P, J], BF16)
                flat_rs = rowsum_dram[:].rearrange("h s -> (h s)") \
                                        .rearrange("(p j) -> p j", p=P)
                nc.sync.dma_start(rs_sprd, flat_rs)
                rinv_sprd = rrepp.tile([P, J], F32)
                nc.vector.reciprocal(rinv_sprd, rs_sprd)
                rinv_dram = dram.tile([NH_core, S], BF16)
                nc.gpsimd.dma_start(
                    rinv_dram[:].rearrange("h s -> (h s)")
                                .rearrange("(p j) -> p j", p=P), rinv_sprd)
                rinv_rep = rrepp.tile([P, NH_core, S], BF16)
                rdap = rinv_dram[:]
                bcast = bass.AP(tensor=rdap.tensor, offset=rdap.offset,
                                ap=[[0, P]] + list(rdap.ap))
                nc.gpsimd.dma_start(rinv_rep, bcast)
                if dbg:
                    nc.gpsimd.dma_start(
                        dbg_ext["d_rinv"][0:1, :],
                        rinv_dram[:].rearrange("h s -> (h s)"))
                dump("d_rinvrep", rinv_rep)
                for h in range(NH_core):
                    hp, hs = divmod(h, 2)
                    hs *= HD
                    nc.vector.tensor_tensor(
                        ctxT[hs:hs + HD, hp, :], ctxT[hs:hs + HD, hp, :],
                        rinv_rep[hs:hs + HD, h, :], ALU.mult)

                if dbg:
                    nc.gpsimd.dma_start(
                        dbg_ext["d_rowsum"][0:1, :],
                        rowsum_dram[:].rearrange("h s -> (h s)"))
                dump("d_ctxT", ctxT)
                # Wo partial: attn[t, H] = ctxT.T @ wo
                wo_sb = wop.tile([P, DT3, H], BF16)
                nc.sync.dma_start(wo_sb, wo.rearrange("(a p) h -> p a h", p=P))
                attn_sb = attnp.tile([P, ST, H], BF16)
                for tt in range(ST):
                    for hoc in range(NHOC):
                        ps_a = wops.tile([P, HOC], F32)
                        for st in range(DT3):
                            nc.tensor.matmul(
                                ps_a,
                                ctxT[:, st, tt * P:(tt + 1) * P],
                                wo_sb[:, st, hoc * HOC:(hoc + 1) * HOC],
                                start=(st == 0), stop=(st == DT3 - 1))
                        nc.vector.tensor_copy(
                            attn_sb[:, tt, hoc * HOC:(hoc + 1) * HOC], ps_a)
                dump("d_attn", attn_sb)
                nc.sync.dma_start(
                    rs_in.rearrange("(tt p) h -> p tt h", p=P), attn_sb[:])

        nc.gpsimd.collective_compute(
            "ReduceScatter", ALU.add, replica_groups=REPLICA_GROUPS,
            ins=[rs_in.opt()], outs=[rs_out.opt()])

        if stage < 4:
            with tc.tile_pool(name=pn("sink3"), bufs=1) as skp:
                sink = skp.tile([P, H], F32)
                nc.sync.dma_start(sink, rs_out[0:P, :])
                nc.sync.dma_start(out_ext[0:P, 0:H], sink)
            return
        # ---------------- phase 3: LN1, FFN, LN2 --------------------------
        with (
            tc.tile_pool(name=pn("x1_keep"), bufs=1) as x1p,
            tc.tile_pool(name=pn("ln_tmp"), bufs=3) as lntp,
            tc.tile_pool(name=pn("ln_ps"), bufs=2, space="PSUM") as lnps_unused,  # noqa
        ):
            if dbg:
                nc.gpsimd.dma_start(dbg_ext["d_rsout"][:], rs_out[:])
            x1 = x1p.tile([P, ST2, H], F32)
            x1T = x1p.tile([P, HT, S2], BF16)
            out2 = x1p.tile([P, ST2, H], F32)

            SG = min(512, H)
            NSG = H // SG

            def layernorm_tile(y_t, out_sl, g_sb, b_sb):
                st6 = lntp.tile([P, NSG, 6], F32, tag="st6")
                for sg in range(NSG):
                    nc.vector.bn_stats(st6[:, sg, :],
                                       y_t[:, sg * SG:(sg + 1) * SG])
                mv = lntp.tile([P, 2], F32, tag="mv")
                nc.vector.bn_aggr(mv, st6)
                nc.scalar.activation(mv[:, 1:2], mv[:, 1:2], AF.Sqrt,
                                     bias=eps_sb)
                nc.vector.reciprocal(mv[:, 1:2], mv[:, 1:2])
                nc.vector.tensor_scalar(out_sl, y_t, mv[:, 0:1], mv[:, 1:2],
                                        ALU.subtract, ALU.mult)
                if g_sb is not None:
                    nc.vector.tensor_tensor(out_sl, out_sl, g_sb, ALU.mult)
                if b_sb is not None:
                    nc.vector.tensor_tensor(out_sl, out_sl, b_sb, ALU.add)

            with tc.tile_pool(name=pn("ln1_tmp"), bufs=3) as ln1tp:
                for tt in range(ST2):
                    xh_t = ln1tp.tile([P, H], F32, tag="xh")
                    nc.sync.dma_start(xh_t, xh[tt * P:(tt + 1) * P, :])
                    at_t = ln1tp.tile([P, H], BF16, tag="at")
                    nc.sync.dma_start(at_t, rs_out[tt * P:(tt + 1) * P, :])
                    y_t = ln1tp.tile([P, H], F32, tag="y")
                    nc.vector.tensor_tensor(y_t, xh_t, at_t, ALU.add)
                    if bo_sb is not None:
                        nc.vector.tensor_tensor(y_t, y_t, bo_sb, ALU.add)
                    layernorm_tile(y_t, x1[:, tt, :], ln1g_sb, ln1b_sb)
                    x1b_t = ln1tp.tile([P, H], BF16, tag="x1b")
                    nc.vector.tensor_copy(x1b_t, x1[:, tt, :])
                    nc.sync.dma_start_transpose(
                        x1T[:, :, tt * P:(tt + 1) * P], x1b_t)

            dump("d_x1", x1)
            dump("d_x1T", x1T)
            with (
                tc.tile_pool(name=pn("ffn_w"), bufs=2) as fwp,
                tc.tile_pool(name=pn("gt"), bufs=2) as gtp,
                tc.tile_pool(name=pn("h_ps"), bufs=4, space="PSUM") as hps,
                tc.tile_pool(name=pn("o_ps"), bufs=4, space="PSUM") as ops,
            ):
                for fq in range(NFQ):
                    f_sl = slice(fq * FQ, (fq + 1) * FQ)
                    w1q = fwp.tile([P, HT, FQ], BF16, tag="w1q")
                    nc.sync.dma_start(
                        w1q, w1[:, f_sl].rearrange("(a p) f -> p a f", p=P))
                    w2q = fwp.tile([P, FTQ, H], BF16, tag="w2q")
                    nc.sync.dma_start(
                        w2q, w2[f_sl, :].rearrange("(a p) h -> p a h", p=P))
                    gt = gtp.tile([P, FTQ, S2], BF16)
                    for ft in range(FTQ):
                        for tci in range(NTC2):
                            ps = hps.tile([P, TC2], F32)
                            for ht in range(HT):
                                nc.tensor.matmul(
                                    ps, w1q[:, ht, ft * P:(ft + 1) * P],
                                    x1T[:, ht, tci * TC2:(tci + 1) * TC2],
                                    start=(ht == 0), stop=(ht == HT - 1))
                            bias = (b1_sb[:, fq * FTQ + ft:fq * FTQ + ft + 1]
                                    if b1_sb is not None else 0.0)
                            nc.scalar.activation(
                                gt[:, ft, tci * TC2:(tci + 1) * TC2], ps,
                                AF.Gelu_apprx_tanh, bias=bias)
                    for tt in range(ST2):
                        for hoc in range(NHOC):
                            o_sl = slice(hoc * HOC, (hoc + 1) * HOC)
                            ps2 = ops.tile([P, HOC], F32)
                            for ft in range(FTQ):
                                nc.tensor.matmul(
                                    ps2, gt[:, ft, tt * P:(tt + 1) * P],
                                    w2q[:, ft, o_sl],
                                    start=(ft == 0), stop=(ft == FTQ - 1))
                            if fq == 0:
                                nc.vector.tensor_copy(
                                    out2[:, tt, o_sl], ps2)
                            else:
                                nc.vector.tensor_tensor(
                                    out2[:, tt, o_sl], out2[:, tt, o_sl],
                                    ps2, ALU.add)

            dump("d_out2", out2)
            with tc.tile_pool(name=pn("ln2_tmp"), bufs=3) as ln2tp:
                for tt in range(ST2):
                    y_t = ln2tp.tile([P, H], F32, tag="y2")
                    nc.vector.tensor_tensor(y_t, out2[:, tt, :], x1[:, tt, :],
                                            ALU.add)
                    if b2_sb is not None:
                        nc.vector.tensor_tensor(y_t, y_t, b2_sb, ALU.add)
                    o_t = ln2tp.tile([P, H], F32, tag="o")
                    layernorm_tile(y_t, o_t, ln2g_sb, ln2b_sb)
                    nc.sync.dma_start(out_ext[tt * P:(tt + 1) * P, :], o_t)


# ---------------------------------------------------------------------------
# host side
# ---------------------------------------------------------------------------

def _nonzero(a):
    return bool(np.any(np.asarray(a) != 0))


def make_in_maps(S, H, NH_core, FF, inputs, flags):
    """Shard full inputs into 8 per-core input maps (big tensors as bf16)."""
    import ml_dtypes
    bf16 = ml_dtypes.bfloat16
    D3 = NH_core * 64
    S2 = S // 2
    x = np.asarray(inputs["x"], np.float32)
    maps = []
    for c in range(8):
        b, j = divmod(c, 2)
        g0 = j * D3
        m = {
            "x": np.ascontiguousarray(x[b]).astype(bf16),
            "xh": np.ascontiguousarray(x[b, j * S2:(j + 1) * S2]),
            "wq": np.ascontiguousarray(inputs["Wq"][:, g0:g0 + D3]).astype(bf16),
            "wk": np.ascontiguousarray(inputs["Wk"][:, g0:g0 + D3]).astype(bf16),
            "wv": np.ascontiguousarray(inputs["Wv"][:, g0:g0 + D3]).astype(bf16),
            "wo": np.ascontiguousarray(inputs["Wo"][g0:g0 + D3, :]).astype(bf16),
            "w1": np.asarray(inputs["W1"], np.float32).astype(bf16),
            "w2": np.asarray(inputs["W2"], np.float32).astype(bf16),
        }
        if "mask" in flags:
            m["mask"] = np.ascontiguousarray(
                np.asarray(inputs["attention_mask"], np.float32)[b, 0, 0, :])
        for src, dst, sl in [("bq", "bq", (g0, D3)), ("bk", "bk", (g0, D3)),
                             ("bv", "bv", (g0, D3))]:
            if dst in flags:
                m[dst] = np.ascontiguousarray(
                    np.asarray(inputs[src], np.float32)[sl[0]:sl[0] + sl[1]])
        for name in ["bo", "b1", "b2", "ln1_g", "ln1_b", "ln2_g", "ln2_b"]:
            if name in flags:
                m[name] = np.asarray(inputs[name], np.float32)
        maps.append({k: (v if v.dtype == bf16 else np.asarray(v, np.float32))
                     for k, v in m.items()})
    return maps


def compute_flags(inputs):
    flags = set()
    if _nonzero(inputs["attention_mask"]):
        flags.add("mask")
    for name in ["bq", "bk", "bv", "bo", "b1", "b2",
                 "ln1_b", "ln2_b"]:
        if _nonzero(inputs[name]):
            flags.add(name)
    for name in ["ln1_g", "ln2_g"]:
        if bool(np.any(np.asarray(inputs[name]) != 1)):
            flags.add(name)
    return flags


LAST_EXEC_NS = None
LAST_RESULTS = None


def run_pjrt_timed(nc, in_maps, n_cores=8, time_iters=0):
    """Like bass2jax.run_bass_via_pjrt (multi-core branch), but jits once and
    optionally times repeated executions. Returns (results, best_exec_s)."""
    import time as _time
    import jax
    from jax.sharding import Mesh, PartitionSpec
    from jax.experimental.shard_map import shard_map
    from concourse import bass2jax, mybir as _mybir

    bass2jax.install_neuronx_cc_hook()
    if nc.dbg_addr is not None:
        assert not nc.dbg_callbacks
        in_maps = [{**m, nc.dbg_addr.name: np.zeros((1, 2), np.uint32)}
                   for m in in_maps]
    partition_name = (nc.partition_id_tensor.name
                      if nc.partition_id_tensor else None)
    in_names, out_names, out_avals, zero_outs = [], [], [], []
    for alloc in nc.m.functions[0].allocations:
        if not isinstance(alloc, _mybir.MemoryLocationSet):
            continue
        name = alloc.memorylocations[0].name
        if alloc.kind == "ExternalInput":
            if name != partition_name:
                in_names.append(name)
        elif alloc.kind == "ExternalOutput":
            shape = tuple(alloc.tensor_shape)
            dtype = _mybir.dt.np(alloc.dtype)
            out_names.append(name)
            out_avals.append(jax.core.ShapedArray(shape, dtype))
            zero_outs.append(np.zeros(shape, dtype))
    n_params = len(in_names)
    n_outs = len(out_avals)
    in_names = in_names + out_names
    if partition_name is not None:
        in_names.append(partition_name)
    donate = tuple(range(n_params, n_params + n_outs))

    def _body(*args):
        operands = list(args)
        if partition_name is not None:
            operands.append(bass2jax.partition_id_tensor())
        return tuple(bass2jax._bass_exec_p.bind(
            *operands, out_avals=tuple(out_avals),
            in_names=tuple(in_names), out_names=tuple(out_names),
            lowering_input_output_aliases=(),
            sim_require_finite=True, sim_require_nnan=True, nc=nc))

    devices = jax.devices()[:n_cores]
    mesh = Mesh(np.asarray(devices), ("core",))
    in_specs = (PartitionSpec("core"),) * (n_params + n_outs)
    out_specs = (PartitionSpec("core"),) * n_outs
    sharded = jax.jit(
        shard_map(_body, mesh=mesh, in_specs=in_specs, out_specs=out_specs,
                  check_rep=False),
        donate_argnums=donate, keep_unused=True)
    per_core = [[np.asarray(m[nm]) for nm in in_names[:n_params]]
                for m in in_maps]
    concat_in = [np.concatenate([per_core[c][i] for c in range(n_cores)],
                                axis=0) for i in range(n_params)]
    concat_zeros = [np.zeros((n_cores * z.shape[0], *z.shape[1:]), z.dtype)
                    for z in zero_outs]
    out_arrs = sharded(*concat_in, *concat_zeros)
    jax.block_until_ready(out_arrs)
    results = [
        {name: np.asarray(out_arrs[i]).reshape(n_cores, *out_avals[i].shape)[c]
         for i, name in enumerate(out_names)}
        for c in range(n_cores)]
    best = None
    if time_iters:
        dev_in = [jax.device_put(a) for a in concat_in]
        jax.block_until_ready(dev_in)
        for _ in range(time_iters):
            zs = [jax.device_put(np.zeros(
                (n_cores * z.shape[0], *z.shape[1:]), z.dtype))
                for z in zero_outs]
            jax.block_until_ready(zs)
            t0 = _time.perf_counter()
            o = sharded(*dev_in, *zs)
            jax.block_until_ready(o)
            dt = _time.perf_counter() - t0
            best = dt if best is None else min(best, dt)
    return results, best


def run_block(S, H, NH_core, FF, inputs, trace=False, dbg=False):
    """Build, compile, run on 8 cores; returns [B, S, H] output."""
    global LAST_EXEC_NS, LAST_RESULTS
    flags = compute_flags(inputs)
    nc = bacc.Bacc("TRN2", target_bir_lowering=False, debug=True)
    build_block(nc, S=S, H=H, NH_core=NH_core, FF=FF, flags=flags, dbg=dbg)
    nc.compile()
    in_maps = make_in_maps(S, H, NH_core, FF, inputs, flags)
    time_iters = int(os.environ.get("BLOCK_TIME_ITERS", "0")) if trace else 0
    results, best_s = run_pjrt_timed(nc, in_maps, n_cores=8,
                                     time_iters=time_iters)
    LAST_EXEC_NS = int(best_s * 1e9) if best_s is not None else None

    class _R:
        pass
    LAST_RESULTS = _R()
    LAST_RESULTS.results = results
    LAST_RESULTS.mean_exec_time_ns = None
    LAST_RESULTS.max_exec_time_core_id = None
    S2 = S // 2
    B = 4
    out = np.empty((B, S, H), np.float32)
    for c in range(8):
        b, j = divmod(c, 2)
        out[b, j * S2:(j + 1) * S2] = results[c]["out"]
    return out


def kernel(x, attention_mask, Wq, bq, Wk, bk, Wv, bv, Wo, bo,
           ln1_g, ln1_b, W1, b1, W2, b2, ln2_g, ln2_b):
    inputs = dict(x=x, attention_mask=attention_mask, Wq=Wq, bq=bq, Wk=Wk,
                  bk=bk, Wv=Wv, bv=bv, Wo=Wo, bo=bo, ln1_g=ln1_g,
                  ln1_b=ln1_b, W1=W1, b1=b1, W2=W2, b2=b2, ln2_g=ln2_g,
                  ln2_b=ln2_b)
    trace = bool(int(os.environ.get("BLOCK_TRACE", "0")))
    return run_block(2048, 1024, 8, 4096, inputs, trace=trace)



# revision 4
# speedup vs baseline: 50.8218x; 50.8218x over previous
"""Transformer block (post-LN, BERT-style) on 8 TRN2 NeuronCores, collective-free.

Sharding: 8 cores = 4 batches x 2 query-halves. Core c=(b,j) computes, for
batch b:
  - K/V projections for all 2048 tokens (recomputed per core pair; cheaper
    than any collective at this size),
  - Q projection + attention + output projection for its own 1024 query
    tokens (all 16 heads),
  - LN1, full FFN, LN2 for its 1024 tokens.
Host concatenates the 8 [1024, 1024] output slices. No collectives.

Layouts keep activations transposed ([feature, token]) so every matmul uses
weights in natural layout; x arrives pre-transposed from the host; softmax
row-sums come from a ones-column appended to V; 1/sqrt(hd) folds into the Q
projection epilogue.
"""

import os
import sys
import types
import numpy as np

import concourse.bacc as bacc
import concourse.bass as bass
import concourse.tile as tile
import concourse.mybir as mybir
from concourse.bass_utils import run_bass_kernel_spmd

P = 128
F32 = mybir.dt.float32
BF16 = mybir.dt.bfloat16
AF = mybir.ActivationFunctionType
ALU = mybir.AluOpType

NH_CORE = 16  # heads per core (all of them; cores split over batch x seq)


def build_block(nc, *, S, H, NH_core, FF, eps=1e-12, flags=None, prefix=""):
    """Emit the SPMD program for one core. flags: set of optional-input names
    among {mask, bq, bk, bv, bo, b1, b2, ln1_g, ln1_b, ln2_g, ln2_b} that are
    actually present (nonzero / non-one)."""
    flags = flags or set()
    HD = 64
    NH = NH_core               # 16 heads, all on this core
    SQ = S // 2                # query tokens owned by this core
    HT = H // P                # 8 feature subtiles of H
    KT = S // P                # 16 k-token tiles
    KG = 4                     # k-tiles per exp batch
    NKG = KT // KG
    QC = 512                   # query chunk (tokens per attention sweep)
    NQC = SQ // QC             # 2
    TC = 512                   # token chunk in projections
    HOC = 512                  # H-output chunk
    NHOC = H // HOC
    NFQ = 4                    # stream FFN weights in quarters
    FQ = FF // NFQ
    FTQ = FQ // P
    TT_Q = QC // P             # 4 token tiles per query chunk

    def pn(n):
        return f"{prefix}{n}"

    def param(name, shape, dt=F32):
        return nc.declare_dram_parameter(pn(name), list(shape), dt,
                                         isOutput=False)

    xT = param("xT", [H, S], BF16)
    xqT = param("xqT", [H, SQ], BF16)
    xh = param("xh", [SQ, H])
    wq = param("wq", [H, H], BF16)
    wk = param("wk", [H, H], BF16)
    wv = param("wv", [H, H], BF16)
    wo = param("wo", [H, H], BF16)
    w1 = param("w1", [H, FF], BF16)
    w2 = param("w2", [FF, H], BF16)
    opt = {}
    for name, shape in [("mask", [S]), ("bq", [H]), ("bk", [H]), ("bv", [H]),
                        ("bo", [H]), ("b1", [FF]), ("b2", [H]),
                        ("ln1_g", [H]), ("ln1_b", [H]),
                        ("ln2_g", [H]), ("ln2_b", [H])]:
        if name in flags:
            opt[name] = param(name, shape)
    out_ext = nc.declare_dram_parameter(pn("out"), [SQ, H], F32, isOutput=True)

    with (
        tile.TileContext(nc) as tc,
        tc.tile_pool(name=pn("singles"), bufs=1) as singles,
        tc.tile_pool(name=pn("dram"), bufs=1, space="DRAM") as dram,
    ):
        eps_sb = singles.tile([P, 1], F32)
        nc.vector.memset(eps_sb, eps)
        mask_sb = None
        if "mask" in flags:
            mask_sb = singles.tile([P, KT], F32)
            nc.gpsimd.dma_start(mask_sb, opt["mask"].rearrange("(a p) -> p a", p=P))

        # per-partition bias strips ([P, n//P]: feature f at [f%P, f//P])
        def col_strip(name, n):
            if name not in flags:
                return None
            t = singles.tile([P, n // P], F32, tag=f"strip_{name}")
            nc.gpsimd.dma_start(t, opt[name].rearrange("(a p) -> p a", p=P))
            return t
        bq_sb = col_strip("bq", H)
        bk_sb = col_strip("bk", H)
        b1_sb = col_strip("b1", FF)

        # partition-replicated rows (for free-dim adds)
        def rep_row(name, n):
            if name not in flags:
                return None
            t = singles.tile([P, n], F32, tag=f"rep_{name}")
            src = opt[name][:]
            bcast = bass.AP(tensor=src.tensor, offset=src.offset,
                            ap=[[0, P]] + list(src.ap))
            nc.gpsimd.dma_start(t, bcast)
            return t
        bv_sb = rep_row("bv", H)
        bo_sb = rep_row("bo", H)
        b2_sb = rep_row("b2", H)
        ln1g_sb = rep_row("ln1_g", H)
        ln1b_sb = rep_row("ln1_b", H)
        ln2g_sb = rep_row("ln2_g", H)
        ln2b_sb = rep_row("ln2_b", H)

        x1_dram = dram.tile([SQ, H], BF16)
        rowsum_dram = dram.tile([NQC, NH, QC], BF16)
        rinv_dram = dram.tile([NQC, NH, QC], BF16)

        SG = 512                      # layernorm bn_stats chunk
        NSG = H // SG

        def layernorm_tile(lntp, y_t, out_sl, g_sb, b_sb):
            st6 = lntp.tile([P, NSG, 6], F32, tag="st6")
            for sg in range(NSG):
                nc.vector.bn_stats(st6[:, sg, :], y_t[:, sg * SG:(sg + 1) * SG])
            mv = lntp.tile([P, 2], F32, tag="mv")
            nc.vector.bn_aggr(mv, st6)
            nc.scalar.activation(mv[:, 1:2], mv[:, 1:2], AF.Sqrt, bias=eps_sb)
            nc.vector.reciprocal(mv[:, 1:2], mv[:, 1:2])
            nc.vector.tensor_scalar(out_sl, y_t, mv[:, 0:1], mv[:, 1:2],
                                    ALU.subtract, ALU.mult)
            if g_sb is not None:
                nc.vector.tensor_tensor(out_sl, out_sl, g_sb, ALU.mult)
            if b_sb is not None:
                nc.vector.tensor_tensor(out_sl, out_sl, b_sb, ALU.add)

        x1T = None
        with tc.tile_pool(name=pn("attn_keep"), bufs=1) as keep:
            qT = keep.tile([P, HT, SQ], BF16)
            kT = keep.tile([P, HT, S], BF16)
            v_sb = keep.tile([P, KT, NH, HD + 1], BF16)
            nc.vector.memset(v_sb[:, :, :, HD:HD + 1], 1.0)

            # ---------------- phase A: projections ------------------------
            with (
                tc.tile_pool(name=pn("qw"), bufs=1) as qwp,
                tc.tile_pool(name=pn("qkv_ps"), bufs=4, space="PSUM") as qps,
            ):
                # Q first: small DMA footprint, warms the PE early.
                xqT_sb = qwp.tile([P, HT, SQ], BF16)
                wq_sb = qwp.tile([P, HT, H], BF16)
                nc.sync.dma_start(xqT_sb, xqT.rearrange("(a p) t -> p a t", p=P))
                nc.sync.dma_start(wq_sb, wq.rearrange("(a p) d -> p a d", p=P))
                for dt in range(HT):
                    for tci in range(SQ // TC):
                        ps = qps.tile([P, TC], F32, tag="qk")
                        for ht in range(HT):
                            nc.tensor.matmul(
                                ps, wq_sb[:, ht, dt * P:(dt + 1) * P],
                                xqT_sb[:, ht, tci * TC:(tci + 1) * TC],
                                start=(ht == 0), stop=(ht == HT - 1))
                        d_sl = qT[:, dt, tci * TC:(tci + 1) * TC]
                        if bq_sb is not None:
                            nc.vector.tensor_scalar(
                                d_sl, ps, bq_sb[:, dt:dt + 1], 0.125,
                                ALU.add, ALU.mult)
                        else:
                            nc.vector.tensor_scalar_mul(d_sl, ps, 0.125)

            with (
                tc.tile_pool(name=pn("kvw"), bufs=1) as kvwp,
                tc.tile_pool(name=pn("kv_ps"), bufs=4, space="PSUM") as kvps,
            ):
                xT_sb = kvwp.tile([P, HT, S], BF16)
                wk_sb = kvwp.tile([P, HT, H], BF16)
                wv_sb = kvwp.tile([P, HT, H], BF16)
                nc.sync.dma_start(xT_sb, xT.rearrange("(a p) t -> p a t", p=P))
                nc.sync.dma_start(wk_sb, wk.rearrange("(a p) d -> p a d", p=P))
                nc.sync.dma_start(wv_sb, wv.rearrange("(a p) d -> p a d", p=P))
                for dt in range(HT):
                    for tci in range(S // TC):
                        ps = kvps.tile([P, TC], F32, tag="k")
                        for ht in range(HT):
                            nc.tensor.matmul(
                                ps, wk_sb[:, ht, dt * P:(dt + 1) * P],
                                xT_sb[:, ht, tci * TC:(tci + 1) * TC],
                                start=(ht == 0), stop=(ht == HT - 1))
                        d_sl = kT[:, dt, tci * TC:(tci + 1) * TC]
                        if bk_sb is not None:
                            nc.vector.tensor_scalar(
                                d_sl, ps, bk_sb[:, dt:dt + 1], 1.0,
                                ALU.add, ALU.mult)
                        else:
                            nc.vector.tensor_copy(d_sl, ps)
                # V: token-major [tok, feat], +bias, ones col at HD
                for tt in range(KT):
                    for dh in range(2):
                        ps = kvps.tile([P, HOC], F32, tag="v")
                        for ht in range(HT):
                            nc.tensor.matmul(
                                ps, xT_sb[:, ht, tt * P:(tt + 1) * P],
                                wv_sb[:, ht, dh * HOC:(dh + 1) * HOC],
                                start=(ht == 0), stop=(ht == HT - 1))
                        if bv_sb is not None:
                            nc.vector.tensor_tensor(
                                ps, ps, bv_sb[:, dh * HOC:(dh + 1) * HOC],
                                ALU.add)
                        nc.vector.tensor_copy(
                            v_sb[:, tt, dh * 8:(dh + 1) * 8, 0:HD],
                            ps.rearrange("p (nh hd) -> p nh hd", hd=HD))

            # ---------------- phase B: attention + Wo + LN1 ---------------
            with (
                tc.tile_pool(name=pn("wo"), bufs=1) as wop,
                tc.tile_pool(name=pn("probs"), bufs=2) as probsp,
                tc.tile_pool(name=pn("stage"), bufs=3) as stagep,
                tc.tile_pool(name=pn("ctxk"), bufs=1) as ctxkp,
                tc.tile_pool(name=pn("rrep"), bufs=1) as rrepp,
                tc.tile_pool(name=pn("ln1"), bufs=2) as ln1p,
                tc.tile_pool(name=pn("sc_ps"), bufs=1, space="PSUM") as scp,
                tc.tile_pool(name=pn("ctx_ps"), bufs=2, space="PSUM") as ctxp,
                tc.tile_pool(name=pn("wo_ps"), bufs=2, space="PSUM") as wops,
            ):
                wo_sb = wop.tile([P, HT, H], BF16)
                nc.sync.dma_start(wo_sb, wo.rearrange("(a p) h -> p a h", p=P))
                for qc in range(NQC):
                    q_sl = slice(qc * QC, (qc + 1) * QC)
                    ctxT = ctxkp.tile([P, HT, QC], BF16, tag="ctxT")
                    for h in range(NH):
                        hp, hs = divmod(h, 2)
                        hs *= HD
                        probs = probsp.tile([P, KT, QC], BF16, tag="probs")
                        for kg in range(NKG):
                            ps_s = scp.tile([P, KG, QC], F32, tag="sc")
                            for j in range(KG):
                                kt = kg * KG + j
                                nc.tensor.matmul(
                                    ps_s[:, j, :],
                                    kT[hs:hs + HD, hp, kt * P:(kt + 1) * P],
                                    qT[hs:hs + HD, hp, q_sl],
                                    start=True, stop=True)
                            if mask_sb is not None:
                                mvw = mask_sb[:, kg * KG:(kg + 1) * KG, None]
                                nc.vector.tensor_tensor(
                                    ps_s, ps_s,
                                    mvw.to_broadcast((P, KG, QC)), ALU.add)
                            nc.scalar.activation(
                                probs[:, kg * KG:(kg + 1) * KG, :], ps_s,
                                AF.Exp)
                        ps_c = ctxp.tile([P, QC], F32, tag="ctx")
                        for kt in range(KT):
                            nc.tensor.matmul(
                                ps_c[0:HD + 1, :],
                                v_sb[:, kt, h, :],
                                probs[:, kt, :],
                                start=(kt == 0), stop=(kt == KT - 1))
                        cs = stagep.tile([P, QC], BF16, tag="cs")
                        nc.vector.tensor_copy(cs[0:HD + 1, :], ps_c[0:HD + 1, :])
                        nc.sync.dma_start(ctxT[hs:hs + HD, hp, :], cs[0:HD, :])
                        nc.sync.dma_start(rowsum_dram[qc, h:h + 1, :],
                                          cs[HD:HD + 1, :])

                    # 1/rowsum, broadcast across partitions via DRAM
                    J = NH * QC // P
                    rs_sprd = rrepp.tile([P, J], BF16, tag="rs")
                    nc.sync.dma_start(
                        rs_sprd,
                        rowsum_dram[qc].rearrange("h s -> (h s)")
                                       .rearrange("(p j) -> p j", p=P))
                    rinv_sprd = rrepp.tile([P, J], F32, tag="rinv")
                    nc.vector.reciprocal(rinv_sprd, rs_sprd)
                    nc.gpsimd.dma_start(
                        rinv_dram[qc].rearrange("h s -> (h s)")
                                     .rearrange("(p j) -> p j", p=P),
                        rinv_sprd)
                    rinv_rep = rrepp.tile([P, NH, QC], BF16, tag="rrep")
                    rdap = rinv_dram[qc]
                    bcast = bass.AP(tensor=rdap.tensor, offset=rdap.offset,
                                    ap=[[0, P]] + list(rdap.ap))
                    nc.gpsimd.dma_start(rinv_rep, bcast)
                    for h in range(NH):
                        hp, hs = divmod(h, 2)
                        hs *= HD
                        nc.vector.tensor_tensor(
                            ctxT[hs:hs + HD, hp, :], ctxT[hs:hs + HD, hp, :],
                            rinv_rep[hs:hs + HD, h, :], ALU.mult)

                    # Wo + residual + LN1 for this query chunk
                    for tt in range(TT_Q):
                        tok0 = qc * QC + tt * P
                        xh_t = ln1p.tile([P, H], F32, tag="xh")
                        nc.sync.dma_start(xh_t, xh[tok0:tok0 + P, :])
                        y_t = ln1p.tile([P, H], F32, tag="y")
                        for hoc in range(NHOC):
                            o_sl = slice(hoc * HOC, (hoc + 1) * HOC)
                            ps_a = wops.tile([P, HOC], F32, tag="wo")
                            for st in range(HT):
                                nc.tensor.matmul(
                                    ps_a,
                                    ctxT[:, st, tt * P:(tt + 1) * P],
                                    wo_sb[:, st, o_sl],
                                    start=(st == 0), stop=(st == HT - 1))
                            nc.vector.tensor_tensor(y_t[:, o_sl], ps_a,
                                                    xh_t[:, o_sl], ALU.add)
                        if bo_sb is not None:
                            nc.vector.tensor_tensor(y_t, y_t, bo_sb, ALU.add)
                        x1b_t = ln1p.tile([P, H], BF16, tag="x1b")
                        layernorm_tile(ln1p, y_t, x1b_t, ln1g_sb, ln1b_sb)
                        nc.sync.dma_start(x1_dram[tok0:tok0 + P, :], x1b_t)

        # ---------------- phase C: FFN + LN2 ------------------------------
        with (
            tc.tile_pool(name=pn("x1keep"), bufs=1) as x1kp,
            tc.tile_pool(name=pn("x1ld"), bufs=3) as x1ldp,
            tc.tile_pool(name=pn("ffn_w"), bufs=2) as fwp,
            tc.tile_pool(name=pn("gt"), bufs=2) as gtp,
            tc.tile_pool(name=pn("out2"), bufs=1) as out2p,
            tc.tile_pool(name=pn("ln2"), bufs=2) as ln2p,
            tc.tile_pool(name=pn("h_ps"), bufs=4, space="PSUM") as hps,
            tc.tile_pool(name=pn("o_ps"), bufs=4, space="PSUM") as ops,
        ):
            ST2 = SQ // P            # 8 token tiles
            x1T = x1kp.tile([P, HT, SQ], BF16)
            for tt in range(ST2):
                x1l = x1ldp.tile([P, H], BF16, tag="x1l")
                nc.sync.dma_start(x1l, x1_dram[tt * P:(tt + 1) * P, :])
                nc.sync.dma_start_transpose(
                    x1T[:, :, tt * P:(tt + 1) * P], x1l)
            out2 = out2p.tile([P, ST2, H], F32)
            for fq in range(NFQ):
                f_sl = slice(fq * FQ, (fq + 1) * FQ)
                w1q = fwp.tile([P, HT, FQ], BF16, tag="w1q")
                nc.sync.dma_start(
                    w1q, w1[:, f_sl].rearrange("(a p) f -> p a f", p=P))
                w2q = fwp.tile([P, FTQ, H], BF16, tag="w2q")
                nc.sync.dma_start(
                    w2q, w2[f_sl, :].rearrange("(a p) h -> p a h", p=P))
                gt = gtp.tile([P, FTQ, SQ], BF16, tag="gt")
                for ft in range(FTQ):
                    for tci in range(SQ // TC):
                        ps = hps.tile([P, TC], F32, tag="h")
                        for ht in range(HT):
                            nc.tensor.matmul(
                                ps, w1q[:, ht, ft * P:(ft + 1) * P],
                                x1T[:, ht, tci * TC:(tci + 1) * TC],
                                start=(ht == 0), stop=(ht == HT - 1))
                        bias = (b1_sb[:, fq * FTQ + ft:fq * FTQ + ft + 1]
                                if b1_sb is not None else 0.0)
                        nc.scalar.activation(
                            gt[:, ft, tci * TC:(tci + 1) * TC], ps,
                            AF.Gelu_apprx_tanh, bias=bias)
                for tt in range(ST2):
                    for hoc in range(NHOC):
                        o_sl = slice(hoc * HOC, (hoc + 1) * HOC)
                        ps2 = ops.tile([P, HOC], F32, tag="o")
                        for ft in range(FTQ):
                            nc.tensor.matmul(
                                ps2, gt[:, ft, tt * P:(tt + 1) * P],
                                w2q[:, ft, o_sl],
                                start=(ft == 0), stop=(ft == FTQ - 1))
                        if fq == 0:
                            nc.vector.tensor_copy(out2[:, tt, o_sl], ps2)
                        else:
                            nc.vector.tensor_tensor(
                                out2[:, tt, o_sl], out2[:, tt, o_sl],
                                ps2, ALU.add)
            for tt in range(ST2):
                x1l2 = ln2p.tile([P, H], BF16, tag="x1l2")
                nc.sync.dma_start(x1l2, x1_dram[tt * P:(tt + 1) * P, :])
                y_t = ln2p.tile([P, H], F32, tag="y2")
                nc.vector.tensor_tensor(y_t, out2[:, tt, :], x1l2, ALU.add)
                if b2_sb is not None:
                    nc.vector.tensor_tensor(y_t, y_t, b2_sb, ALU.add)
                o_t = ln2p.tile([P, H], F32, tag="o")
                layernorm_tile(ln2p, y_t, o_t, ln2g_sb, ln2b_sb)
                nc.sync.dma_start(out_ext[tt * P:(tt + 1) * P, :], o_t)


# ---------------------------------------------------------------------------
# host side
# ---------------------------------------------------------------------------

def _nonzero(a):
    return bool(np.any(np.asarray(a) != 0))


def compute_flags(inputs):
    flags = set()
    if _nonzero(inputs["attention_mask"]):
        flags.add("mask")
    for name in ["bq", "bk", "bv", "bo", "b1", "b2", "ln1_b", "ln2_b"]:
        if _nonzero(inputs[name]):
            flags.add(name)
    for name in ["ln1_g", "ln2_g"]:
        if bool(np.any(np.asarray(inputs[name]) != 1)):
            flags.add(name)
    return flags


def make_in_maps(S, H, FF, inputs, flags):
    """Shard full inputs into 8 per-core input maps (big tensors as bf16)."""
    import ml_dtypes
    bf16 = ml_dtypes.bfloat16
    SQ = S // 2
    x = np.asarray(inputs["x"], np.float32)       # [4, S, H]
    shared = {
        "wq": np.ascontiguousarray(np.asarray(inputs["Wq"], np.float32)).astype(bf16),
        "wk": np.ascontiguousarray(np.asarray(inputs["Wk"], np.float32)).astype(bf16),
        "wv": np.ascontiguousarray(np.asarray(inputs["Wv"], np.float32)).astype(bf16),
        "wo": np.ascontiguousarray(np.asarray(inputs["Wo"], np.float32)).astype(bf16),
        "w1": np.ascontiguousarray(np.asarray(inputs["W1"], np.float32)).astype(bf16),
        "w2": np.ascontiguousarray(np.asarray(inputs["W2"], np.float32)).astype(bf16),
    }
    for name in ["bq", "bk", "bv", "bo", "b1", "b2",
                 "ln1_g", "ln1_b", "ln2_g", "ln2_b"]:
        if name in flags:
            src = {"bq": "bq", "bk": "bk", "bv": "bv", "bo": "bo",
                   "b1": "b1", "b2": "b2", "ln1_g": "ln1_g", "ln1_b": "ln1_b",
                   "ln2_g": "ln2_g", "ln2_b": "ln2_b"}[name]
            shared[name] = np.ascontiguousarray(
                np.asarray(inputs[src], np.float32))
    xT_by_batch = [np.ascontiguousarray(x[b].T).astype(bf16) for b in range(4)]
    maps = []
    for c in range(8):
        b, j = divmod(c, 2)
        xTb = xT_by_batch[b]
        m = dict(shared)
        m["xT"] = xTb
        m["xqT"] = np.ascontiguousarray(xTb[:, j * SQ:(j + 1) * SQ])
        m["xh"] = np.ascontiguousarray(x[b, j * SQ:(j + 1) * SQ])
        if "mask" in flags:
            m["mask"] = np.ascontiguousarray(
                np.asarray(inputs["attention_mask"], np.float32)[b, 0, 0, :])
        maps.append(m)
    return maps


LAST_EXEC_NS = None
LAST_RESULTS = None


def _install_ntff_hook():
    """Register the NTFF profiling hook (missing antenv.axon_hooks shim)."""
    if "antenv.axon_hooks" in sys.modules:
        return
    try:
        import antenv  # noqa: F401
        mod = types.ModuleType("antenv.axon_hooks")
        hook = [None]
        mod.set_axon_ntff_profile_hook = lambda h: hook.__setitem__(0, h)
        mod.get_axon_ntff_profile_hook = lambda: hook[0]
        sys.modules["antenv.axon_hooks"] = mod
        from trn_agent_boot.trn_boot import _ntff_profile_via_ctypes
        mod.set_axon_ntff_profile_hook(
            _ntff_profile_via_ctypes("/opt/axon/libaxon_pjrt.so"))
    except Exception:
        sys.modules.pop("antenv.axon_hooks", None)


def run_block(S, H, FF, inputs, trace=False):
    """Build, compile, run on 8 cores; returns [B, S, H] output."""
    global LAST_EXEC_NS, LAST_RESULTS
    flags = compute_flags(inputs)
    nc = bacc.Bacc("TRN2", target_bir_lowering=False, debug=True)
    build_block(nc, S=S, H=H, NH_core=NH_CORE, FF=FF, flags=flags)
    nc.compile()
    in_maps = make_in_maps(S, H, FF, inputs, flags)
    if trace:
        _install_ntff_hook()
    res = run_bass_kernel_spmd(
        nc, in_maps, core_ids=list(range(8)), trace=trace,
        trace_cores=[0] if trace else None)
    LAST_EXEC_NS = res.exec_time_ns
    LAST_RESULTS = res
    SQ = S // 2
    B = 4
    out = np.empty((B, S, H), np.float32)
    for c in range(8):
        b, j = divmod(c, 2)
        out[b, j * SQ:(j + 1) * SQ] = res.results[c]["out"]
    return out


def kernel(x, attention_mask, Wq, bq, Wk, bk, Wv, bv, Wo, bo,
           ln1_g, ln1_b, W1, b1, W2, b2, ln2_g, ln2_b):
    inputs = dict(x=x, attention_mask=attention_mask, Wq=Wq, bq=bq, Wk=Wk,
                  bk=bk, Wv=Wv, bv=bv, Wo=Wo, bo=bo, ln1_g=ln1_g,
                  ln1_b=ln1_b, W1=W1, b1=b1, W2=W2, b2=b2, ln2_g=ln2_g,
                  ln2_b=ln2_b)
    trace = bool(int(os.environ.get("BLOCK_TRACE", "0")))
    return run_block(2048, 1024, 4096, inputs, trace=trace)


# revision 16
# speedup vs baseline: 54.5599x; 1.0736x over previous
"""Transformer block (post-LN, BERT-style) on 8 TRN2 NeuronCores, collective-free.

Sharding: 8 cores = 4 batches x 2 query-halves. Core c=(b,j) computes, for
batch b:
  - K/V projections for all 2048 tokens (recomputed per core pair; cheaper
    than any collective at this size),
  - Q projection + attention + output projection for its own 1024 query
    tokens (all 16 heads),
  - LN1, full FFN, LN2 for its 1024 tokens.
Host concatenates the 8 [1024, 1024] output slices. No collectives.

Layouts keep activations transposed ([feature, token]) so every matmul uses
weights in natural layout; x arrives pre-transposed from the host; softmax
row-sums come from a ones-column appended to V; 1/sqrt(hd) folds into the Q
projection epilogue.
"""

import os
import sys
import types
import numpy as np

import concourse.bacc as bacc
import concourse.bass as bass
import concourse.tile as tile
import concourse.mybir as mybir
from concourse.bass_utils import run_bass_kernel_spmd

P = 128
F32 = mybir.dt.float32
BF16 = mybir.dt.bfloat16
AF = mybir.ActivationFunctionType
ALU = mybir.AluOpType

NH_CORE = 16  # heads per core (all of them; cores split over batch x seq)


def build_block(nc, *, S, H, NH_core, FF, eps=1e-12, flags=None, prefix=""):
    """Emit the SPMD program for one core. flags: set of optional-input names
    among {mask, bq, bk, bv, bo, b1, b2, ln1_g, ln1_b, ln2_g, ln2_b} that are
    actually present (nonzero / non-one)."""
    flags = flags or set()
    HD = 64
    NH = NH_core               # 16 heads, all on this core
    SQ = S // 2                # query tokens owned by this core
    HT = H // P                # 8 feature subtiles of H
    KT = S // P                # 16 k-token tiles
    KG = 2                     # k-tiles per exp batch (2 PSUM banks/slot)
    NKG = KT // KG
    QC = 512                   # query chunk (tokens per attention sweep)
    NQC = SQ // QC             # 2
    TC = 512                   # token chunk in projections
    HOC = 512                  # H-output chunk
    NHOC = H // HOC
    NFQ = 4                    # stream FFN weights in quarters
    FQ = FF // NFQ
    FTQ = FQ // P
    TT_Q = QC // P             # 4 token tiles per query chunk

    def pn(n):
        return f"{prefix}{n}"

    def param(name, shape, dt=F32):
        return nc.declare_dram_parameter(pn(name), list(shape), dt,
                                         isOutput=False)

    xT = param("xT", [H, S], BF16)
    xqT = param("xqT", [H, SQ], BF16)
    xh = param("xh", [SQ, H])
    wq = param("wq", [H, H], BF16)
    wk = param("wk", [H, H], BF16)
    wv = param("wv", [H, H], BF16)
    wo = param("wo", [H, H], BF16)
    w1 = param("w1", [H, FF], BF16)
    w2 = param("w2", [FF, H], BF16)
    opt = {}
    for name, shape in [("mask", [S]), ("bq", [H]), ("bk", [H]), ("bv", [H]),
                        ("bo", [H]), ("b1", [FF]), ("b2", [H]),
                        ("ln1_g", [H]), ("ln1_b", [H]),
                        ("ln2_g", [H]), ("ln2_b", [H])]:
        if name in flags:
            opt[name] = param(name, shape)
    out_ext = nc.declare_dram_parameter(pn("out"), [SQ, H], F32, isOutput=True)

    with (
        tile.TileContext(nc) as tc,
        tc.tile_pool(name=pn("singles"), bufs=1) as singles,
        tc.tile_pool(name=pn("dram"), bufs=1, space="DRAM") as dram,
    ):
        eps_sb = singles.tile([P, 1], F32)
        nc.vector.memset(eps_sb, eps)
        mask_sb = None
        if "mask" in flags:
            mask_sb = singles.tile([P, KT], F32)
            nc.gpsimd.dma_start(mask_sb, opt["mask"].rearrange("(a p) -> p a", p=P))

        # per-partition bias strips ([P, n//P]: feature f at [f%P, f//P])
        def col_strip(name, n):
            if name not in flags:
                return None
            t = singles.tile([P, n // P], F32, tag=f"strip_{name}")
            nc.gpsimd.dma_start(t, opt[name].rearrange("(a p) -> p a", p=P))
            return t
        bq_sb = col_strip("bq", H)
        bk_sb = col_strip("bk", H)
        b1_sb = col_strip("b1", FF)

        # partition-replicated rows (for free-dim adds)
        def rep_row(name, n):
            if name not in flags:
                return None
            t = singles.tile([P, n], F32, tag=f"rep_{name}")
            src = opt[name][:]
            bcast = bass.AP(tensor=src.tensor, offset=src.offset,
                            ap=[[0, P]] + list(src.ap))
            nc.gpsimd.dma_start(t, bcast)
            return t
        bv_sb = rep_row("bv", H)
        bo_sb = rep_row("bo", H)
        b2_sb = rep_row("b2", H)
        ln1g_sb = rep_row("ln1_g", H)
        ln1b_sb = rep_row("ln1_b", H)
        ln2g_sb = rep_row("ln2_g", H)
        ln2b_sb = rep_row("ln2_b", H)

        ones_sb = singles.tile([P, HD], F32)
        nc.vector.memset(ones_sb, 1.0)
        x1_dram = dram.tile([SQ, H], BF16)

        SG = 512                      # layernorm bn_stats chunk
        NSG = H // SG

        def layernorm_tile(lntp, y_t, out_sl, g_sb, b_sb):
            st6 = lntp.tile([P, NSG, 6], F32, tag="st6")
            for sg in range(NSG):
                nc.vector.bn_stats(st6[:, sg, :], y_t[:, sg * SG:(sg + 1) * SG])
            mv = lntp.tile([P, 2], F32, tag="mv")
            nc.vector.bn_aggr(mv, st6)
            nc.scalar.activation(mv[:, 1:2], mv[:, 1:2], AF.Sqrt, bias=eps_sb)
            nc.vector.reciprocal(mv[:, 1:2], mv[:, 1:2])
            nc.vector.tensor_scalar(out_sl, y_t, mv[:, 0:1], mv[:, 1:2],
                                    ALU.subtract, ALU.mult)
            if g_sb is not None:
                nc.vector.tensor_tensor(out_sl, out_sl, g_sb, ALU.mult)
            if b_sb is not None:
                nc.vector.tensor_tensor(out_sl, out_sl, b_sb, ALU.add)

        ST2 = SQ // P            # 8 token tiles
        x1T = singles.tile([P, HT, SQ], BF16)
        with tc.tile_pool(name=pn("attn_keep"), bufs=1) as keep:
            qT = keep.tile([P, HT, SQ], BF16)
            kT = keep.tile([P, HT, S], BF16)
            v_sb = keep.tile([P, KT, NH, HD + 1], BF16)
            nc.vector.memset(v_sb[:, :, :, HD:HD + 1], 1.0)

            # ---------------- phase A: projections ------------------------
            with (
                tc.tile_pool(name=pn("qw"), bufs=1) as qwp,
                tc.tile_pool(name=pn("qkv_ps"), bufs=4, space="PSUM") as qps,
            ):
                # Q first: small DMA footprint, warms the PE early.
                xqT_sb = qwp.tile([P, HT, SQ], BF16)
                wq_sb = qwp.tile([P, HT, H], BF16)
                nc.sync.dma_start(xqT_sb, xqT.rearrange("(a p) t -> p a t", p=P))
                nc.sync.dma_start(wq_sb, wq.rearrange("(a p) d -> p a d", p=P))
                for dt in range(HT):
                    for tci in range(SQ // TC):
                        ps = qps.tile([P, TC], F32, tag="qk")
                        for ht in range(HT):
                            nc.tensor.matmul(
                                ps, wq_sb[:, ht, dt * P:(dt + 1) * P],
                                xqT_sb[:, ht, tci * TC:(tci + 1) * TC],
                                start=(ht == 0), stop=(ht == HT - 1))
                        d_sl = qT[:, dt, tci * TC:(tci + 1) * TC]
                        if bq_sb is not None:
                            nc.vector.tensor_scalar(
                                d_sl, ps, bq_sb[:, dt:dt + 1], 0.125,
                                ALU.add, ALU.mult)
                        else:
                            nc.vector.tensor_scalar_mul(d_sl, ps, 0.125)

            with (
                tc.tile_pool(name=pn("kvw"), bufs=1) as kvwp,
                tc.tile_pool(name=pn("xtc"), bufs=2) as xtcp,
                tc.tile_pool(name=pn("kv_ps"), bufs=4, space="PSUM") as kvps,
            ):
                wk_sb = kvwp.tile([P, HT, H], BF16)
                wv_sb = kvwp.tile([P, HT, H], BF16)
                # scalar-queue DMAs run in parallel with the sync-queue loads
                # above, so kT can start right after the Q matmuls.
                nc.scalar.dma_start(wk_sb, wk.rearrange("(a p) d -> p a d", p=P))
                nc.scalar.dma_start(wv_sb, wv.rearrange("(a p) d -> p a d", p=P))
                xTr = xT.rearrange("(a p) t -> p a t", p=P)
                for tci in range(S // TC):
                    t_sl = slice(tci * TC, (tci + 1) * TC)
                    xT_c = xtcp.tile([P, HT, TC], BF16, tag="xc")
                    nc.scalar.dma_start(xT_c, xTr[:, :, t_sl])
                    for dt in range(HT):
                        ps = kvps.tile([P, TC], F32, tag="k")
                        for ht in range(HT):
                            nc.tensor.matmul(
                                ps, wk_sb[:, ht, dt * P:(dt + 1) * P],
                                xT_c[:, ht, :],
                                start=(ht == 0), stop=(ht == HT - 1))
                        d_sl = kT[:, dt, t_sl]
                        if bk_sb is not None:
                            nc.vector.tensor_scalar(
                                d_sl, ps, bk_sb[:, dt:dt + 1], 1.0,
                                ALU.add, ALU.mult)
                        else:
                            nc.vector.tensor_copy(d_sl, ps)
                    # V for this token chunk: token-major, ones col at HD
                    for lt in range(TC // P):
                        tt = tci * (TC // P) + lt
                        for dh in range(2):
                            ps = kvps.tile([P, HOC], F32, tag="v")
                            for ht in range(HT):
                                nc.tensor.matmul(
                                    ps, xT_c[:, ht, lt * P:(lt + 1) * P],
                                    wv_sb[:, ht, dh * HOC:(dh + 1) * HOC],
                                    start=(ht == 0), stop=(ht == HT - 1))
                            if bv_sb is not None:
                                nc.vector.tensor_tensor(
                                    ps, ps, bv_sb[:, dh * HOC:(dh + 1) * HOC],
                                    ALU.add)
                            nc.vector.tensor_copy(
                                v_sb[:, tt, dh * 8:(dh + 1) * 8, 0:HD],
                                ps.rearrange("p (nh hd) -> p nh hd", hd=HD))

            # ---------------- phase B: attention + Wo + LN1 ---------------
            with (
                tc.tile_pool(name=pn("wo"), bufs=1) as wop,
                tc.tile_pool(name=pn("probs"), bufs=2) as probsp,
                tc.tile_pool(name=pn("stage"), bufs=3) as stagep,
                tc.tile_pool(name=pn("ctxk"), bufs=2) as ctxkp,
                tc.tile_pool(name=pn("ln1"), bufs=2) as ln1p,
                tc.tile_pool(name=pn("sc_ps"), bufs=2, space="PSUM") as scp,
                tc.tile_pool(name=pn("ctx_ps"), bufs=2, space="PSUM") as ctxp,
                tc.tile_pool(name=pn("wo_ps"), bufs=2, space="PSUM") as wops,
            ):
                wo_sb = wop.tile([P, HT, H], BF16)
                nc.sync.dma_start(wo_sb, wo.rearrange("(a p) h -> p a h", p=P))
                for qc in range(NQC):
                    q_sl = slice(qc * QC, (qc + 1) * QC)
                    ctxT = ctxkp.tile([P, HT, QC], BF16, tag="ctxT")
                    for h in range(NH):
                        hp, hs = divmod(h, 2)
                        hs *= HD
                        probs = probsp.tile([P, KT, QC], BF16, tag="probs")
                        for kg in range(NKG):
                            ps_s = scp.tile([P, KG, QC], F32, tag="sc")
                            for j in range(KG):
                                kt = kg * KG + j
                                nc.tensor.matmul(
                                    ps_s[:, j, :],
                                    kT[hs:hs + HD, hp, kt * P:(kt + 1) * P],
                                    qT[hs:hs + HD, hp, q_sl],
                                    start=True, stop=True)
                            if mask_sb is not None:
                                mvw = mask_sb[:, kg * KG:(kg + 1) * KG, None]
                                nc.vector.tensor_tensor(
                                    ps_s, ps_s,
                                    mvw.to_broadcast((P, KG, QC)), ALU.add)
                            nc.scalar.activation(
                                probs[:, kg * KG:(kg + 1) * KG, :], ps_s,
                                AF.Exp)
                        ps_c = ctxp.tile([P, QC], F32, tag="ctx")
                        for kt in range(KT):
                            nc.tensor.matmul(
                                ps_c[0:HD + 1, :],
                                v_sb[:, kt, h, :],
                                probs[:, kt, :],
                                start=(kt == 0), stop=(kt == KT - 1))
                        # softmax normalize in place: 1/rowsum broadcast over
                        # the 64 hd partitions via a K=1 fp32 matmul, then
                        # fused into the PSUM->SBUF copy.
                        cs = stagep.tile([P, QC], BF16, tag="cs")
                        rr = stagep.tile([P, QC], F32, tag="rr")
                        nc.vector.reciprocal(rr[HD:HD + 1, :],
                                             ps_c[HD:HD + 1, :])
                        rb = wops.tile([P, QC], F32, tag="rbwo")
                        nc.tensor.matmul(rb[0:HD, :],
                                         ones_sb[HD:HD + 1, :],
                                         rr[HD:HD + 1, :],
                                         start=True, stop=True)
                        nc.vector.tensor_copy(cs[0:HD, :], ps_c[0:HD, :])
                        nc.vector.tensor_tensor(cs[0:HD, :], cs[0:HD, :],
                                                rb[0:HD, :], ALU.mult)
                        nc.sync.dma_start(ctxT[hs:hs + HD, hp, :], cs[0:HD, :])

                    # Wo + residual + LN1 for this query chunk
                    for tt in range(TT_Q):
                        tok0 = qc * QC + tt * P
                        xh_t = ln1p.tile([P, H], F32, tag="xh")
                        nc.sync.dma_start(xh_t, xh[tok0:tok0 + P, :])
                        y_t = ln1p.tile([P, H], F32, tag="y")
                        for hoc in range(NHOC):
                            o_sl = slice(hoc * HOC, (hoc + 1) * HOC)
                            ps_a = wops.tile([P, HOC], F32, tag="rbwo")
                            for st in range(HT):
                                nc.tensor.matmul(
                                    ps_a,
                                    ctxT[:, st, tt * P:(tt + 1) * P],
                                    wo_sb[:, st, o_sl],
                                    start=(st == 0), stop=(st == HT - 1))
                            nc.vector.tensor_tensor(y_t[:, o_sl], ps_a,
                                                    xh_t[:, o_sl], ALU.add)
                        if bo_sb is not None:
                            nc.vector.tensor_tensor(y_t, y_t, bo_sb, ALU.add)
                        x1b_t = ln1p.tile([P, H], BF16, tag="x1b")
                        layernorm_tile(ln1p, y_t, x1b_t, ln1g_sb, ln1b_sb)
                        nc.sync.dma_start(x1_dram[tok0:tok0 + P, :], x1b_t)
                        nc.sync.dma_start_transpose(
                            x1T[:, :, tok0:tok0 + P], x1b_t)

        # ---------------- phase C: FFN + LN2 ------------------------------
        with (
            tc.tile_pool(name=pn("ffn_w"), bufs=2) as fwp,
            tc.tile_pool(name=pn("gt"), bufs=2) as gtp,
            tc.tile_pool(name=pn("out2"), bufs=1) as out2p,
            tc.tile_pool(name=pn("ln2"), bufs=2) as ln2p,
            tc.tile_pool(name=pn("h_ps"), bufs=4, space="PSUM") as hps,
            tc.tile_pool(name=pn("o_ps"), bufs=4, space="PSUM") as ops,
        ):
            out2 = out2p.tile([P, ST2, H], F32)
            for fq in range(NFQ):
                f_sl = slice(fq * FQ, (fq + 1) * FQ)
                w1q = fwp.tile([P, HT, FQ], BF16, tag="w1q")
                nc.sync.dma_start(
                    w1q, w1[:, f_sl].rearrange("(a p) f -> p a f", p=P))
                w2q = fwp.tile([P, FTQ, H], BF16, tag="w2q")
                nc.sync.dma_start(
                    w2q, w2[f_sl, :].rearrange("(a p) h -> p a h", p=P))
                gt = gtp.tile([P, FTQ, SQ], BF16, tag="gt")
                for ft in range(FTQ):
                    for tci in range(SQ // TC):
                        ps = hps.tile([P, TC], F32, tag="h")
                        for ht in range(HT):
                            nc.tensor.matmul(
                                ps, w1q[:, ht, ft * P:(ft + 1) * P],
                                x1T[:, ht, tci * TC:(tci + 1) * TC],
                                start=(ht == 0), stop=(ht == HT - 1))
                        bias = (b1_sb[:, fq * FTQ + ft:fq * FTQ + ft + 1]
                                if b1_sb is not None else 0.0)
                        nc.scalar.activation(
                            gt[:, ft, tci * TC:(tci + 1) * TC], ps,
                            AF.Gelu_apprx_tanh, bias=bias)
                for tt in range(ST2):
                    for hoc in range(NHOC):
                        o_sl = slice(hoc * HOC, (hoc + 1) * HOC)
                        ps2 = ops.tile([P, HOC], F32, tag="o")
                        for ft in range(FTQ):
                            nc.tensor.matmul(
                                ps2, gt[:, ft, tt * P:(tt + 1) * P],
                                w2q[:, ft, o_sl],
                                start=(ft == 0), stop=(ft == FTQ - 1))
                        if fq == 0:
                            nc.vector.tensor_copy(out2[:, tt, o_sl], ps2)
                        else:
                            nc.vector.tensor_tensor(
                                out2[:, tt, o_sl], out2[:, tt, o_sl],
                                ps2, ALU.add)
            for tt in range(ST2):
                x1l2 = ln2p.tile([P, H], BF16, tag="x1l2")
                nc.sync.dma_start(x1l2, x1_dram[tt * P:(tt + 1) * P, :])
                y_t = ln2p.tile([P, H], F32, tag="y2")
                nc.vector.tensor_tensor(y_t, out2[:, tt, :], x1l2, ALU.add)
                if b2_sb is not None:
                    nc.vector.tensor_tensor(y_t, y_t, b2_sb, ALU.add)
                o_t = ln2p.tile([P, H], F32, tag="o")
                layernorm_tile(ln2p, y_t, o_t, ln2g_sb, ln2b_sb)
                nc.sync.dma_start(out_ext[tt * P:(tt + 1) * P, :], o_t)


# ---------------------------------------------------------------------------
# host side
# ---------------------------------------------------------------------------

def _nonzero(a):
    return bool(np.any(np.asarray(a) != 0))


def compute_flags(inputs):
    flags = set()
    if _nonzero(inputs["attention_mask"]):
        flags.add("mask")
    for name in ["bq", "bk", "bv", "bo", "b1", "b2", "ln1_b", "ln2_b"]:
        if _nonzero(inputs[name]):
            flags.add(name)
    for name in ["ln1_g", "ln2_g"]:
        if bool(np.any(np.asarray(inputs[name]) != 1)):
            flags.add(name)
    return flags


def make_in_maps(S, H, FF, inputs, flags):
    """Shard full inputs into 8 per-core input maps (big tensors as bf16)."""
    import ml_dtypes
    bf16 = ml_dtypes.bfloat16
    SQ = S // 2
    x = np.asarray(inputs["x"], np.float32)       # [4, S, H]
    shared = {
        "wq": np.ascontiguousarray(np.asarray(inputs["Wq"], np.float32)).astype(bf16),
        "wk": np.ascontiguousarray(np.asarray(inputs["Wk"], np.float32)).astype(bf16),
        "wv": np.ascontiguousarray(np.asarray(inputs["Wv"], np.float32)).astype(bf16),
        "wo": np.ascontiguousarray(np.asarray(inputs["Wo"], np.float32)).astype(bf16),
        "w1": np.ascontiguousarray(np.asarray(inputs["W1"], np.float32)).astype(bf16),
        "w2": np.ascontiguousarray(np.asarray(inputs["W2"], np.float32)).astype(bf16),
    }
    for name in ["bq", "bk", "bv", "bo", "b1", "b2",
                 "ln1_g", "ln1_b", "ln2_g", "ln2_b"]:
        if name in flags:
            src = {"bq": "bq", "bk": "bk", "bv": "bv", "bo": "bo",
                   "b1": "b1", "b2": "b2", "ln1_g": "ln1_g", "ln1_b": "ln1_b",
                   "ln2_g": "ln2_g", "ln2_b": "ln2_b"}[name]
            shared[name] = np.ascontiguousarray(
                np.asarray(inputs[src], np.float32))
    xT_by_batch = [np.ascontiguousarray(x[b].T).astype(bf16) for b in range(4)]
    maps = []
    for c in range(8):
        b, j = divmod(c, 2)
        xTb = xT_by_batch[b]
        m = dict(shared)
        m["xT"] = xTb
        m["xqT"] = np.ascontiguousarray(xTb[:, j * SQ:(j + 1) * SQ])
        m["xh"] = np.ascontiguousarray(x[b, j * SQ:(j + 1) * SQ])
        if "mask" in flags:
            m["mask"] = np.ascontiguousarray(
                np.asarray(inputs["attention_mask"], np.float32)[b, 0, 0, :])
        maps.append(m)
    return maps


LAST_EXEC_NS = None
LAST_RESULTS = None


def _install_ntff_hook():
    """Register the NTFF profiling hook (missing antenv.axon_hooks shim)."""
    if "antenv.axon_hooks" in sys.modules:
        return
    try:
        import antenv  # noqa: F401
        mod = types.ModuleType("antenv.axon_hooks")
        hook = [None]
        mod.set_axon_ntff_profile_hook = lambda h: hook.__setitem__(0, h)
        mod.get_axon_ntff_profile_hook = lambda: hook[0]
        sys.modules["antenv.axon_hooks"] = mod
        from trn_agent_boot.trn_boot import _ntff_profile_via_ctypes
        mod.set_axon_ntff_profile_hook(
            _ntff_profile_via_ctypes("/opt/axon/libaxon_pjrt.so"))
    except Exception:
        sys.modules.pop("antenv.axon_hooks", None)


def run_block(S, H, FF, inputs, trace=False):
    """Build, compile, run on 8 cores; returns [B, S, H] output."""
    global LAST_EXEC_NS, LAST_RESULTS
    flags = compute_flags(inputs)
    nc = bacc.Bacc("TRN2", target_bir_lowering=False, debug=True)
    build_block(nc, S=S, H=H, NH_core=NH_CORE, FF=FF, flags=flags)
    nc.compile()
    in_maps = make_in_maps(S, H, FF, inputs, flags)
    if trace:
        _install_ntff_hook()
    res = run_bass_kernel_spmd(
        nc, in_maps, core_ids=list(range(8)), trace=trace,
        trace_cores=[0] if trace else None)
    LAST_EXEC_NS = res.exec_time_ns
    LAST_RESULTS = res
    SQ = S // 2
    B = 4
    out = np.empty((B, S, H), np.float32)
    for c in range(8):
        b, j = divmod(c, 2)
        out[b, j * SQ:(j + 1) * SQ] = res.results[c]["out"]
    return out


def kernel(x, attention_mask, Wq, bq, Wk, bk, Wv, bv, Wo, bo,
           ln1_g, ln1_b, W1, b1, W2, b2, ln2_g, ln2_b):
    inputs = dict(x=x, attention_mask=attention_mask, Wq=Wq, bq=bq, Wk=Wk,
                  bk=bk, Wv=Wv, bv=bv, Wo=Wo, bo=bo, ln1_g=ln1_g,
                  ln1_b=ln1_b, W1=W1, b1=b1, W2=W2, b2=b2, ln2_g=ln2_g,
                  ln2_b=ln2_b)
    trace = bool(int(os.environ.get("BLOCK_TRACE", "0")))
    return run_block(2048, 1024, 4096, inputs, trace=trace)
